# revision 35
# baseline (speedup 1.0000x reference)
"""Causal self-attention (B=4, T=2048, C=1024, NH=16) on 8 TRN2 NeuronCores.

Sharding: core c -> batch b = c//2, head-group g = c%2 (8 heads, Dh=512).
Each core computes q/k/v projections for its head group on its batch,
fused causal attention (attT layout: k on partitions), and a partial
output projection through its row-slice of Wp. Host sums the two
partials per batch.

Pipeline structure (per 512-row block ib): projections for block ib ->
attention stage jq=ib over all 4 head pairs -> normalization ->
output projection + store for block ib. The Tile list scheduler fills
PE bubbles of the ACT-bound attention chain with projection/out-proj
matmuls of neighboring stages.

Per-core dataflow: everything bf16 on the matmul paths. QK for the
even/odd head of a pair go to the two banks of one [128,1024] psum
tile (row groups 0-1 / 2-3 run concurrently) so ONE activation
instruction exps both. v is stored per-k-chunk as AV lhsT slots with a
ones column that makes softmax denominators a free extra psum row of
the AV matmul. Causal: only lower-triangle k-chunks computed; QK, exp
and AV all trim to the valid q columns; diagonal chunks masked with a
host tri mask after exp. Normalization: s rows collected on 8
partitions of a tiny tile -> one Ln + one Exp -> K=8 PE broadcast
matmul -> one scale per (hp, block).

kernel(**inputs) takes the FULL unsharded inputs and returns the FULL
output. Self-contained: hardcodes all shapes, reads nothing from disk.
"""

import sys

sys.path.insert(0, "/opt/trn_rl_repo")

import numpy as np
import ml_dtypes
from contextlib import ExitStack

import concourse.bass as bass  # noqa: F401  (engine types referenced via nc)
import concourse.mybir as mybir
import concourse.tile as tile
from concourse import bacc
from concourse.bass_utils import run_bass_kernel_spmd

P = 128
B, T, C = 4, 2048, 1024
NH, HS = 16, 64
D = 512          # per-core head dim (8 heads)
H = 8            # local heads
f32 = mybir.dt.float32
bf16 = mybir.dt.bfloat16
AFT = mybir.ActivationFunctionType


def build_nc(t=T):
    """Build the single-core SPMD program (same code, per-core data)."""
    assert t % 512 == 0
    nb = t // 512    # 512-row blocks (proj blocks == q blocks == stages)
    nkc = t // 128   # k chunks of 128
    nco = C // P     # C chunks (8)

    nc = bacc.Bacc("TRN2", target_bir_lowering=False, debug=False, num_devices=8)

    xt_d = nc.dram_tensor("xt", [C, t], bf16, kind="ExternalInput")
    wq_d = nc.dram_tensor("wq", [C, D], bf16, kind="ExternalInput")
    wk_d = nc.dram_tensor("wk", [C, D], bf16, kind="ExternalInput")
    wv_d = nc.dram_tensor("wv", [C, D], bf16, kind="ExternalInput")
    wp_d = nc.dram_tensor("wp", [D, C], bf16, kind="ExternalInput")
    tri_d = nc.dram_tensor("tri", [P, 2, P], bf16, kind="ExternalInput")
    bcm_d = nc.dram_tensor("bcm", [P, 4, P], bf16, kind="ExternalInput")
    out_d = nc.dram_tensor("out", [t, C], bf16, kind="ExternalOutput")

    xt_r = xt_d[:].rearrange("(co p) t -> p co t", p=P)
    wq_r = wq_d[:].rearrange("(co p) d -> p co d", p=P)
    wk_r = wk_d[:].rearrange("(co p) d -> p co d", p=P)
    wv_r = wv_d[:].rearrange("(co p) d -> p co d", p=P)
    wp_r = wp_d[:].rearrange("(dc p) c -> p dc c", p=P)
    out_r = out_d[:].rearrange("(tc p) c -> p tc c", p=P)

    with tile.TileContext(nc) as tc, ExitStack() as ctx, nc.allow_low_precision(
        reason="bf16 attention kernel"
    ):
        perm = ctx.enter_context(tc.tile_pool(name="perm", bufs=1))
        work = ctx.enter_context(tc.tile_pool(name="work", bufs=1))
        psum = ctx.enter_context(tc.tile_pool(name="psum", bufs=1, space="PSUM"))

        # per-block tiles so stage ib's reads and proj ib+1's writes are
        # on different tiles (no false WAR edges to serialize stages)
        qt_t = [perm.tile([P, 4, 512], bf16, name=f"qt{i}") for i in range(nb)]
        kt_t = [perm.tile([P, 4, 512], bf16, name=f"kt{i}") for i in range(nb)]
        v_t = [perm.tile([P, 4, H, P], bf16, name=f"v{i}") for i in range(nb)]
        yt_t = [perm.tile([P, 4, 512], bf16, name=f"yt{i}") for i in range(nb)]
        wq_sb = perm.tile([P, nco, D], bf16)
        wk_sb = perm.tile([P, nco, D], bf16)
        wv_sb = perm.tile([P, nco, D], bf16)
        wp_sb = perm.tile([P, 4, C], bf16)
        tri_sb = perm.tile([P, 2, P], bf16)
        bcm_sb = perm.tile([P, 4, P], bf16)
        # s rows: even head of hp at [64, hp, :], odd at [0, hp, :].
        # Junk partitions 1:64 stay 1.0 so the reciprocal keeps them
        # finite for the K=65 broadcast matmul.
        scol = perm.tile([P, 4, 512], f32)
        scolf = perm.tile([P, 4, 512], f32)
        scolr_t = [perm.tile([P, 4, 512], bf16, name=f"scolr{i}")
                   for i in range(nb)]

        # DMA order: first q-proj matmul needs only chunk 0 of wq and x
        xtb0 = work.tile([P, nco, 512], bf16, tag="xtb", bufs=2)
        nc.sync.dma_start(wq_sb[:, 0, :], wq_r[:, 0, :])
        nc.sync.dma_start(xtb0[:, 0, :], xt_r[:, 0, 0:512])
        nc.sync.dma_start(wq_sb[:, 1:nco, :], wq_r[:, 1:nco, :])
        nc.sync.dma_start(xtb0[:, 1:nco, :], xt_r[:, 1:nco, 0:512])
        nc.sync.dma_start(wk_sb[:], wk_r)
        nc.sync.dma_start(wv_sb[:], wv_r)
        nc.sync.dma_start(tri_sb[:], tri_d[:])
        nc.sync.dma_start(bcm_sb[:], bcm_d[:])
        nc.sync.dma_start(wp_sb[:], wp_r)
        # zero v slots (junk cols would put NaNs in unread psum rows);
        # ones columns: even head -> col 64 (sum row 64); odd -> col 0 (row 0)
        v5s = []
        for vb in v_t:
            nc.gpsimd.memset(vb[:], 0.0)
            v5 = vb[:].rearrange("p k (hp par) c -> p k hp par c", par=2)
            nc.gpsimd.memset(v5[:, :, :, 0, 64:65], 1.0)
            nc.gpsimd.memset(v5[:, :, :, 1, 0:1], 1.0)
            v5s.append(v5)
        nc.gpsimd.memset(scol[:], 1.0)

        def emit_proj(ibb, xtbb):
            for m in range(4):  # qt/kt row chunks of Dh
                psq = psum.tile([P, 512], f32, tag="gen", bufs=2, name="psq")
                for co in range(nco):
                    nc.tensor.matmul(
                        psq[:],
                        wq_sb[:, co, m * P : (m + 1) * P],
                        xtbb[:, co, :],
                        start=(co == 0),
                        stop=(co == nco - 1),
                    )
                nc.scalar.copy(out=qt_t[ibb][:, m, :], in_=psq[:])
                psk = psum.tile([P, 512], f32, tag="gen", bufs=2, name="psk")
                for co in range(nco):
                    nc.tensor.matmul(
                        psk[:],
                        wk_sb[:, co, m * P : (m + 1) * P],
                        xtbb[:, co, :],
                        start=(co == 0),
                        stop=(co == nco - 1),
                    )
                nc.scalar.copy(out=kt_t[ibb][:, m, :], in_=psk[:])
            for t4 in range(4):  # v chunks of 128 rows within this block
                psv = psum.tile([P, 512], f32, tag="gen", bufs=2, name="psv")
                for co in range(nco):
                    nc.tensor.matmul(
                        psv[:],
                        xtbb[:, co, t4 * P : (t4 + 1) * P],
                        wv_sb[:, co, :],
                        start=(co == 0),
                        stop=(co == nco - 1),
                    )
                # scatter heads into lhsT slots: even -> cols 0:64 of
                # slot (par 0), odd -> cols 64:128 (par 1)
                src = psv[:].rearrange("p (hp par c) -> p hp par c", par=2, c=64)
                nc.vector.tensor_copy(
                    out=v5s[ibb][:, t4, :, 0, 0:64], in_=src[:, :, 0, :]
                )
                nc.vector.tensor_copy(
                    out=v5s[ibb][:, t4, :, 1, 64:128], in_=src[:, :, 1, :]
                )

        lo, hi = slice(0, 64), slice(64, 128)

        def emit_att(ibb, fills=()):
            # fills: thunks emitting one deferred out-proj chunk each,
            # injected mid-chain and at head-pair ends so the in-order PE
            # works through them inside this stage's exp-bound stalls
            fills = list(fills)
            nk = (ibb + 1) * 4
            for hp in range(4):
                pav = psum.tile([P, 1024], f32, tag="av", bufs=1, name="pav")
                for kc in range(nk):
                    d = kc - ibb * 4
                    off = 128 * d if d >= 0 else 0
                    ktb, kcl = kt_t[kc // 4], kc % 4
                    pqk = psum.tile([P, 1024], f32, tag="qk", bufs=2, name="pqk")
                    nc.tensor.matmul(
                        pqk[:, off:512],
                        ktb[lo, hp, kcl * P : (kcl + 1) * P],
                        qt_t[ibb][lo, hp, off:512],
                        start=True,
                        stop=True,
                    )
                    nc.tensor.matmul(
                        pqk[:, 512 + off : 1024],
                        ktb[hi, hp, kcl * P : (kcl + 1) * P],
                        qt_t[ibb][hi, hp, off:512],
                        start=True,
                        stop=True,
                    )
                    att = work.tile([P, 1024], bf16, tag="att", bufs=6, name="att")
                    # one exp over both heads; [512, 512+off) is unwritten
                    # psum junk but lands in att cols the trimmed AV never
                    # reads
                    nc.scalar.activation(
                        att[:, off:1024], pqk[:, off:1024], AFT.Exp, scale=0.125
                    )
                    if d >= 0:
                        # one masked mul covers both heads' diagonal blocks
                        # ([P,2,128] view, stride 512); gpsimd keeps it off
                        # the busier DVE
                        att2v = att[:].rearrange("p (two half) -> p two half",
                                                 two=2)
                        nc.gpsimd.tensor_mul(
                            out=att2v[:, :, off : off + P],
                            in0=att2v[:, :, off : off + P],
                            in1=tri_sb[:],
                        )
                    nc.tensor.matmul(
                        pav[:, off:512],
                        v_t[kc // 4][:, kc % 4, 2 * hp, :],
                        att[:, off:512],
                        start=(kc == 0),
                        stop=(kc == nk - 1),
                    )
                    nc.tensor.matmul(
                        pav[:, 512 + off : 1024],
                        v_t[kc // 4][:, kc % 4, 2 * hp + 1, :],
                        att[:, 512 + off : 1024],
                        start=(kc == 0),
                        stop=(kc == nk - 1),
                    )
                    if fills and kc == nk // 2:
                        fills.pop(0)()
                # yt (unnormalized) + s rows out of the psum halves
                nc.vector.tensor_copy(out=yt_t[ibb][lo, hp, :], in_=pav[0:64, 0:512])
                nc.vector.tensor_copy(
                    out=yt_t[ibb][hi, hp, :], in_=pav[64:128, 512:1024]
                )
                nc.vector.tensor_copy(out=scol[64:65, hp, :], in_=pav[64:65, 0:512])
                nc.vector.tensor_copy(out=scol[0:1, hp, :], in_=pav[0:1, 512:1024])
                if fills:
                    fills.pop(0)()

        def emit_recip(ibb):
            # 1/s on DVE; one pass over partitions 0:65 covers both s rows.
            # Off the ACT engine entirely so its table set never leaves exp.
            # Emitted BEFORE the next block's projections so it sits in the
            # DVE queue ahead of their psum casts.
            nc.vector.reciprocal_approx_fast(
                out=scolf[0:65, :, :], in_=scol[0:65, :, :]
            )
            nc.scalar.copy(out=scolr_t[ibb][0:65, :, :], in_=scolf[0:65, :, :])

        def emit_normmul(ibb):
            for hp in range(4):
                rb = psum.tile([P, 1024], f32, tag="qk", bufs=2, name="rb")
                nc.tensor.matmul(
                    rb[:, 0:512], bcm_sb[0:65, hp, :], scolr_t[ibb][0:65, hp, :],
                    start=True, stop=True,
                )
                nc.vector.tensor_mul(
                    out=yt_t[ibb][:, hp, :],
                    in0=yt_t[ibb][:, hp, :],
                    in1=rb[:, 0:512],
                )

        def emit_outproj_tcn(ibb, t4):
            # single 128-row out-proj chunk on gen psum slots (free during
            # attention stages) so it never steals the QK double-buffer
            tcn = ibb * 4 + t4
            ob = work.tile([P, C], bf16, tag="ob", bufs=2, name="ob")
            for n2 in range(C // 512):
                pso = psum.tile([P, 512], f32, tag="gen", bufs=2, name="psog")
                for dc in range(4):
                    nc.tensor.matmul(
                        pso[:],
                        yt_t[ibb][:, dc, t4 * P : (t4 + 1) * P],
                        wp_sb[:, dc, n2 * 512 : (n2 + 1) * 512],
                        start=(dc == 0),
                        stop=(dc == 3),
                    )
                nc.vector.tensor_copy(
                    out=ob[:, n2 * 512 : (n2 + 1) * 512], in_=pso[:]
                )
            nc.sync.dma_start(out_r[:, tcn, :], ob[:])

        def emit_outproj(ibb):
            for t4 in range(4):
                tcn = ibb * 4 + t4
                ob = work.tile([P, C], bf16, tag="ob", bufs=2, name="ob")
                pso = psum.tile([P, 1024], f32, tag="qk", bufs=2, name="pso")
                for n2 in range(C // 512):
                    for dc in range(4):
                        nc.tensor.matmul(
                            pso[:, n2 * 512 : (n2 + 1) * 512],
                            yt_t[ibb][:, dc, t4 * P : (t4 + 1) * P],
                            wp_sb[:, dc, n2 * 512 : (n2 + 1) * 512],
                            start=(dc == 0),
                            stop=(dc == 3),
                        )
                nc.vector.tensor_copy(out=ob[:], in_=pso[:])
                nc.sync.dma_start(out_r[:, tcn, :], ob[:])

        # Emission order per stage: attention(ib), then proj(ib+1) so the
        # in-order PE crunches projections while the DVE copy/recip chain
        # finishes, then norm(ib) (ready by then) and out-proj. Blocks 1,2
        # of out-proj are deferred into stage 3's window.
        emit_proj(0, xtb0)
        for ib in range(nb):
            if ib + 1 < nb:
                xtb_next = work.tile([P, nco, 512], bf16, tag="xtb", bufs=2,
                                     name="xtb")
                nc.sync.dma_start(
                    xtb_next[:], xt_r[:, :, (ib + 1) * 512 : (ib + 2) * 512]
                )
            if ib == nb - 1:
                emit_normmul(1)
                emit_normmul(2)
                fills = [
                    (lambda jb=jb, t4=t4: emit_outproj_tcn(jb, t4))
                    for jb in (1, 2) for t4 in range(4)
                ]
            else:
                fills = ()
            emit_att(ib, fills)
            emit_recip(ib)
            if ib + 1 < nb:
                emit_proj(ib + 1, xtb_next)
            if ib == 0:
                emit_normmul(0)
                emit_outproj(0)
            elif ib == nb - 1:
                emit_normmul(3)
                emit_outproj(3)

    nc.finalize()
    return nc


_NC = None


def _get_nc():
    global _NC
    if _NC is None:
        _NC = build_nc()
    return _NC


def make_in_maps(x, Wk, Wq, Wv, Wp):
    x = np.asarray(x, dtype=np.float32)
    Wk = np.asarray(Wk, dtype=np.float32)
    Wq = np.asarray(Wq, dtype=np.float32)
    Wv = np.asarray(Wv, dtype=np.float32)
    Wp = np.asarray(Wp, dtype=np.float32)
    tri = np.broadcast_to(
        np.triu(np.ones((P, P), np.float32))[:, None, :], (P, 2, P)
    ).copy().astype(ml_dtypes.bfloat16)
    # bcm broadcasts 1/s rows (even at 64, odd at 0) to yt rows
    bcm = np.zeros((P, 4, P), np.float32)
    for hp in range(4):
        bcm[64, hp, 0:64] = 1.0
        bcm[0, hp, 64:128] = 1.0
    bcm = bcm.astype(ml_dtypes.bfloat16)
    in_maps = []
    for c in range(8):
        b, g = c // 2, c % 2
        sl = slice(g * D, (g + 1) * D)
        in_maps.append({
            "xt": np.ascontiguousarray(x[b].T).astype(ml_dtypes.bfloat16),
            "wq": np.ascontiguousarray(Wq[:, sl]).astype(ml_dtypes.bfloat16),
            "wk": np.ascontiguousarray(Wk[:, sl]).astype(ml_dtypes.bfloat16),
            "wv": np.ascontiguousarray(Wv[:, sl]).astype(ml_dtypes.bfloat16),
            "wp": np.ascontiguousarray(Wp[sl, :]).astype(ml_dtypes.bfloat16),
            "tri": tri,
            "bcm": bcm,
        })
    return in_maps


def _run(x, Wk, Wq, Wv, Wp, trace=False):
    nc = _get_nc()
    in_maps = make_in_maps(x, Wk, Wq, Wv, Wp)
    res = run_bass_kernel_spmd(nc, in_maps, core_ids=list(range(8)), trace=trace)
    parts = [np.asarray(res.results[c]["out"], dtype=np.float32) for c in range(8)]
    out = np.stack(
        [parts[2 * b] + parts[2 * b + 1] for b in range(B)], axis=0
    )
    return out, res


def kernel(x, Wk, Wq, Wv, Wp):
    out, _ = _run(x, Wk, Wq, Wv, Wp, trace=False)
    return out


# revision 36
# speedup vs baseline: 1.0190x; 1.0190x over previous
"""Causal self-attention (B=4, T=2048, C=1024, NH=16) on 8 TRN2 NeuronCores.

Sharding: core c -> batch b = c//2, head-group g = c%2 (8 heads, Dh=512).
Each core computes q/k/v projections for its head group on its batch,
fused causal attention (attT layout: k on partitions), and a partial
output projection through its row-slice of Wp. Host sums the two
partials per batch.

Pipeline structure (per 512-row block ib): projections for block ib ->
attention stage jq=ib over all 4 head pairs -> normalization ->
output projection + store for block ib. The Tile list scheduler fills
PE bubbles of the ACT-bound attention chain with projection/out-proj
matmuls of neighboring stages.

Per-core dataflow: everything bf16 on the matmul paths. QK for the
even/odd head of a pair go to the two banks of one [128,1024] psum
tile (row groups 0-1 / 2-3 run concurrently) so ONE activation
instruction exps both. v is stored per-k-chunk as AV lhsT slots with a
ones column that makes softmax denominators a free extra psum row of
the AV matmul. Causal: only lower-triangle k-chunks computed; QK, exp
and AV all trim to the valid q columns; diagonal chunks masked with a
host tri mask after exp. Normalization: s rows collected on 8
partitions of a tiny tile -> one Ln + one Exp -> K=8 PE broadcast
matmul -> one scale per (hp, block).

kernel(**inputs) takes the FULL unsharded inputs and returns the FULL
output. Self-contained: hardcodes all shapes, reads nothing from disk.
"""

import sys

sys.path.insert(0, "/opt/trn_rl_repo")

import numpy as np
import ml_dtypes
from contextlib import ExitStack

import concourse.bass as bass  # noqa: F401  (engine types referenced via nc)
import concourse.mybir as mybir
import concourse.tile as tile
from concourse import bacc
from concourse.bass_utils import run_bass_kernel_spmd

P = 128
B, T, C = 4, 2048, 1024
NH, HS = 16, 64
D = 512          # per-core head dim (8 heads)
H = 8            # local heads
f32 = mybir.dt.float32
bf16 = mybir.dt.bfloat16
AFT = mybir.ActivationFunctionType


def build_nc(t=T):
    """Build the single-core SPMD program (same code, per-core data)."""
    assert t % 512 == 0
    nb = t // 512    # 512-row blocks (proj blocks == q blocks == stages)
    nkc = t // 128   # k chunks of 128
    nco = C // P     # C chunks (8)

    nc = bacc.Bacc("TRN2", target_bir_lowering=False, debug=False, num_devices=8)

    xt_d = nc.dram_tensor("xt", [C, t], bf16, kind="ExternalInput")
    wq_d = nc.dram_tensor("wq", [C, D], bf16, kind="ExternalInput")
    wk_d = nc.dram_tensor("wk", [C, D], bf16, kind="ExternalInput")
    wv_d = nc.dram_tensor("wv", [C, D], bf16, kind="ExternalInput")
    wp_d = nc.dram_tensor("wp", [D, C], bf16, kind="ExternalInput")
    tri_d = nc.dram_tensor("tri", [P, 2, P], bf16, kind="ExternalInput")
    bcm_d = nc.dram_tensor("bcm", [P, 4, P], bf16, kind="ExternalInput")
    out_d = nc.dram_tensor("out", [t, C], bf16, kind="ExternalOutput")

    xt_r = xt_d[:].rearrange("(co p) t -> p co t", p=P)
    wq_r = wq_d[:].rearrange("(co p) d -> p co d", p=P)
    wk_r = wk_d[:].rearrange("(co p) d -> p co d", p=P)
    wv_r = wv_d[:].rearrange("(co p) d -> p co d", p=P)
    wp_r = wp_d[:].rearrange("(dc p) c -> p dc c", p=P)
    out_r = out_d[:].rearrange("(tc p) c -> p tc c", p=P)

    with tile.TileContext(nc) as tc, ExitStack() as ctx, nc.allow_low_precision(
        reason="bf16 attention kernel"
    ):
        perm = ctx.enter_context(tc.tile_pool(name="perm", bufs=1))
        work = ctx.enter_context(tc.tile_pool(name="work", bufs=1))
        psum = ctx.enter_context(tc.tile_pool(name="psum", bufs=1, space="PSUM"))

        # per-block tiles so stage ib's reads and proj ib+1's writes are
        # on different tiles (no false WAR edges to serialize stages)
        qt_t = [perm.tile([P, 4, 512], bf16, name=f"qt{i}") for i in range(nb)]
        kt_t = [perm.tile([P, 4, 512], bf16, name=f"kt{i}") for i in range(nb)]
        v_t = [perm.tile([P, 4, H, P], bf16, name=f"v{i}") for i in range(nb)]
        yt_t = [perm.tile([P, 4, 512], bf16, name=f"yt{i}") for i in range(nb)]
        wq_sb = perm.tile([P, nco, D], bf16)
        wk_sb = perm.tile([P, nco, D], bf16)
        wv_sb = perm.tile([P, nco, D], bf16)
        wp_sb = perm.tile([P, 4, C], bf16)
        tri_sb = perm.tile([P, 2, P], bf16)
        bcm_sb = perm.tile([P, 4, P], bf16)
        # s rows: even head of hp at [64, hp, :], odd at [0, hp, :].
        # Junk partitions 1:64 stay 1.0 so the reciprocal keeps them
        # finite for the K=65 broadcast matmul.
        scol = perm.tile([P, 4, 512], f32)
        scolf = perm.tile([P, 4, 512], f32)
        scolr_t = [perm.tile([P, 4, 512], bf16, name=f"scolr{i}")
                   for i in range(nb)]

        # DMA order: first q-proj matmul needs only chunk 0 of wq and x
        xtb0 = work.tile([P, nco, 512], bf16, tag="xtb", bufs=2)
        nc.sync.dma_start(wq_sb[:, 0, :], wq_r[:, 0, :])
        nc.sync.dma_start(xtb0[:, 0, :], xt_r[:, 0, 0:512])
        nc.sync.dma_start(wq_sb[:, 1:nco, :], wq_r[:, 1:nco, :])
        nc.sync.dma_start(xtb0[:, 1:nco, :], xt_r[:, 1:nco, 0:512])
        nc.sync.dma_start(wk_sb[:], wk_r)
        nc.sync.dma_start(wv_sb[:], wv_r)
        nc.sync.dma_start(tri_sb[:], tri_d[:])
        nc.sync.dma_start(bcm_sb[:], bcm_d[:])
        nc.sync.dma_start(wp_sb[:], wp_r)
        # zero v slots (junk cols would put NaNs in unread psum rows);
        # ones columns: even head -> col 64 (sum row 64); odd -> col 0 (row 0)
        v5s = []
        for vb in v_t:
            nc.gpsimd.memset(vb[:], 0.0)
            v5 = vb[:].rearrange("p k (hp par) c -> p k hp par c", par=2)
            nc.gpsimd.memset(v5[:, :, :, 0, 64:65], 1.0)
            nc.gpsimd.memset(v5[:, :, :, 1, 0:1], 1.0)
            v5s.append(v5)
        nc.gpsimd.memset(scol[:], 1.0)

        def emit_proj(ibb, xtbb):
            for m in range(4):  # qt/kt row chunks of Dh
                psq = psum.tile([P, 512], f32, tag="gen", bufs=2, name="psq")
                for co in range(nco):
                    nc.tensor.matmul(
                        psq[:],
                        wq_sb[:, co, m * P : (m + 1) * P],
                        xtbb[:, co, :],
                        start=(co == 0),
                        stop=(co == nco - 1),
                    )
                nc.vector.tensor_copy(out=qt_t[ibb][:, m, :], in_=psq[:])
                psk = psum.tile([P, 512], f32, tag="gen", bufs=2, name="psk")
                for co in range(nco):
                    nc.tensor.matmul(
                        psk[:],
                        wk_sb[:, co, m * P : (m + 1) * P],
                        xtbb[:, co, :],
                        start=(co == 0),
                        stop=(co == nco - 1),
                    )
                nc.vector.tensor_copy(out=kt_t[ibb][:, m, :], in_=psk[:])
            for t4 in range(4):  # v chunks of 128 rows within this block
                psv = psum.tile([P, 512], f32, tag="gen", bufs=2, name="psv")
                for co in range(nco):
                    nc.tensor.matmul(
                        psv[:],
                        xtbb[:, co, t4 * P : (t4 + 1) * P],
                        wv_sb[:, co, :],
                        start=(co == 0),
                        stop=(co == nco - 1),
                    )
                # scatter heads into lhsT slots: even -> cols 0:64 of
                # slot (par 0), odd -> cols 64:128 (par 1)
                src = psv[:].rearrange("p (hp par c) -> p hp par c", par=2, c=64)
                nc.vector.tensor_copy(
                    out=v5s[ibb][:, t4, :, 0, 0:64], in_=src[:, :, 0, :]
                )
                nc.vector.tensor_copy(
                    out=v5s[ibb][:, t4, :, 1, 64:128], in_=src[:, :, 1, :]
                )

        lo, hi = slice(0, 64), slice(64, 128)

        def emit_att(ibb, fills=()):
            # fills: thunks emitting one deferred out-proj chunk each,
            # injected mid-chain and at head-pair ends so the in-order PE
            # works through them inside this stage's exp-bound stalls
            fills = list(fills)
            nk = (ibb + 1) * 4
            for hp in range(4):
                pav = psum.tile([P, 1024], f32, tag="av", bufs=1, name="pav")
                for kc in range(nk):
                    d = kc - ibb * 4
                    off = 128 * d if d >= 0 else 0
                    ktb, kcl = kt_t[kc // 4], kc % 4
                    pqk = psum.tile([P, 1024], f32, tag="qk", bufs=2, name="pqk")
                    nc.tensor.matmul(
                        pqk[:, off:512],
                        ktb[lo, hp, kcl * P : (kcl + 1) * P],
                        qt_t[ibb][lo, hp, off:512],
                        start=True,
                        stop=True,
                    )
                    nc.tensor.matmul(
                        pqk[:, 512 + off : 1024],
                        ktb[hi, hp, kcl * P : (kcl + 1) * P],
                        qt_t[ibb][hi, hp, off:512],
                        start=True,
                        stop=True,
                    )
                    att = work.tile([P, 1024], bf16, tag="att", bufs=6, name="att")
                    # one exp over both heads; [512, 512+off) is unwritten
                    # psum junk but lands in att cols the trimmed AV never
                    # reads
                    nc.scalar.activation(
                        att[:, off:1024], pqk[:, off:1024], AFT.Exp, scale=0.125
                    )
                    if d >= 0:
                        # one masked mul covers both heads' diagonal blocks
                        # ([P,2,128] view, stride 512); gpsimd keeps it off
                        # the busier DVE
                        att2v = att[:].rearrange("p (two half) -> p two half",
                                                 two=2)
                        nc.gpsimd.tensor_mul(
                            out=att2v[:, :, off : off + P],
                            in0=att2v[:, :, off : off + P],
                            in1=tri_sb[:],
                        )
                    nc.tensor.matmul(
                        pav[:, off:512],
                        v_t[kc // 4][:, kc % 4, 2 * hp, :],
                        att[:, off:512],
                        start=(kc == 0),
                        stop=(kc == nk - 1),
                    )
                    nc.tensor.matmul(
                        pav[:, 512 + off : 1024],
                        v_t[kc // 4][:, kc % 4, 2 * hp + 1, :],
                        att[:, 512 + off : 1024],
                        start=(kc == 0),
                        stop=(kc == nk - 1),
                    )
                    if fills and kc == nk // 2:
                        fills.pop(0)()
                # yt (unnormalized) + s rows out of the psum halves
                nc.vector.tensor_copy(out=yt_t[ibb][lo, hp, :], in_=pav[0:64, 0:512])
                nc.vector.tensor_copy(
                    out=yt_t[ibb][hi, hp, :], in_=pav[64:128, 512:1024]
                )
                nc.vector.tensor_copy(out=scol[64:65, hp, :], in_=pav[64:65, 0:512])
                nc.vector.tensor_copy(out=scol[0:1, hp, :], in_=pav[0:1, 512:1024])
                if fills:
                    fills.pop(0)()

        def emit_recip(ibb):
            # 1/s on DVE; one pass over partitions 0:65 covers both s rows.
            # Off the ACT engine entirely so its table set never leaves exp.
            # Emitted BEFORE the next block's projections so it sits in the
            # DVE queue ahead of their psum casts.
            nc.vector.reciprocal_approx_fast(
                out=scolf[0:65, :, :], in_=scol[0:65, :, :]
            )
            nc.vector.tensor_copy(
                out=scolr_t[ibb][0:65, :, :], in_=scolf[0:65, :, :]
            )

        def emit_normmul(ibb):
            for hp in range(4):
                rb = psum.tile([P, 1024], f32, tag="qk", bufs=2, name="rb")
                nc.tensor.matmul(
                    rb[:, 0:512], bcm_sb[0:65, hp, :], scolr_t[ibb][0:65, hp, :],
                    start=True, stop=True,
                )
                nc.vector.tensor_mul(
                    out=yt_t[ibb][:, hp, :],
                    in0=yt_t[ibb][:, hp, :],
                    in1=rb[:, 0:512],
                )

        def emit_outproj_tcn(ibb, t4):
            # single 128-row out-proj chunk on gen psum slots (free during
            # attention stages) so it never steals the QK double-buffer
            tcn = ibb * 4 + t4
            ob = work.tile([P, C], bf16, tag="ob", bufs=2, name="ob")
            for n2 in range(C // 512):
                pso = psum.tile([P, 512], f32, tag="gen", bufs=2, name="psog")
                for dc in range(4):
                    nc.tensor.matmul(
                        pso[:],
                        yt_t[ibb][:, dc, t4 * P : (t4 + 1) * P],
                        wp_sb[:, dc, n2 * 512 : (n2 + 1) * 512],
                        start=(dc == 0),
                        stop=(dc == 3),
                    )
                nc.vector.tensor_copy(
                    out=ob[:, n2 * 512 : (n2 + 1) * 512], in_=pso[:]
                )
            nc.sync.dma_start(out_r[:, tcn, :], ob[:])

        def emit_outproj(ibb):
            for t4 in range(4):
                tcn = ibb * 4 + t4
                ob = work.tile([P, C], bf16, tag="ob", bufs=2, name="ob")
                pso = psum.tile([P, 1024], f32, tag="qk", bufs=2, name="pso")
                for n2 in range(C // 512):
                    for dc in range(4):
                        nc.tensor.matmul(
                            pso[:, n2 * 512 : (n2 + 1) * 512],
                            yt_t[ibb][:, dc, t4 * P : (t4 + 1) * P],
                            wp_sb[:, dc, n2 * 512 : (n2 + 1) * 512],
                            start=(dc == 0),
                            stop=(dc == 3),
                        )
                nc.vector.tensor_copy(out=ob[:], in_=pso[:])
                nc.sync.dma_start(out_r[:, tcn, :], ob[:])

        # Emission order per stage: attention(ib), then proj(ib+1) so the
        # in-order PE crunches projections while the DVE copy/recip chain
        # finishes, then norm(ib) (ready by then) and out-proj. Blocks 1,2
        # of out-proj are deferred into stage 3's window.
        emit_proj(0, xtb0)
        for ib in range(nb):
            if ib + 1 < nb:
                xtb_next = work.tile([P, nco, 512], bf16, tag="xtb", bufs=2,
                                     name="xtb")
                nc.sync.dma_start(
                    xtb_next[:], xt_r[:, :, (ib + 1) * 512 : (ib + 2) * 512]
                )
            if ib == nb - 1:
                emit_normmul(1)
                emit_normmul(2)
                fills = [
                    (lambda jb=jb, t4=t4: emit_outproj_tcn(jb, t4))
                    for jb in (1, 2) for t4 in range(4)
                ]
            else:
                fills = ()
            emit_att(ib, fills)
            emit_recip(ib)
            if ib + 1 < nb:
                emit_proj(ib + 1, xtb_next)
            if ib == 0:
                emit_normmul(0)
                emit_outproj(0)
            elif ib == nb - 1:
                emit_normmul(3)
                emit_outproj(3)

    nc.finalize()
    return nc


_NC = None


def _get_nc():
    global _NC
    if _NC is None:
        _NC = build_nc()
    return _NC


def make_in_maps(x, Wk, Wq, Wv, Wp):
    x = np.asarray(x, dtype=np.float32)
    Wk = np.asarray(Wk, dtype=np.float32)
    Wq = np.asarray(Wq, dtype=np.float32)
    Wv = np.asarray(Wv, dtype=np.float32)
    Wp = np.asarray(Wp, dtype=np.float32)
    tri = np.broadcast_to(
        np.triu(np.ones((P, P), np.float32))[:, None, :], (P, 2, P)
    ).copy().astype(ml_dtypes.bfloat16)
    # bcm broadcasts 1/s rows (even at 64, odd at 0) to yt rows
    bcm = np.zeros((P, 4, P), np.float32)
    for hp in range(4):
        bcm[64, hp, 0:64] = 1.0
        bcm[0, hp, 64:128] = 1.0
    bcm = bcm.astype(ml_dtypes.bfloat16)
    in_maps = []
    for c in range(8):
        b, g = c // 2, c % 2
        sl = slice(g * D, (g + 1) * D)
        in_maps.append({
            "xt": np.ascontiguousarray(x[b].T).astype(ml_dtypes.bfloat16),
            "wq": np.ascontiguousarray(Wq[:, sl]).astype(ml_dtypes.bfloat16),
            "wk": np.ascontiguousarray(Wk[:, sl]).astype(ml_dtypes.bfloat16),
            "wv": np.ascontiguousarray(Wv[:, sl]).astype(ml_dtypes.bfloat16),
            "wp": np.ascontiguousarray(Wp[sl, :]).astype(ml_dtypes.bfloat16),
            "tri": tri,
            "bcm": bcm,
        })
    return in_maps


def _run(x, Wk, Wq, Wv, Wp, trace=False):
    nc = _get_nc()
    in_maps = make_in_maps(x, Wk, Wq, Wv, Wp)
    res = run_bass_kernel_spmd(nc, in_maps, core_ids=list(range(8)), trace=trace)
    parts = [np.asarray(res.results[c]["out"], dtype=np.float32) for c in range(8)]
    out = np.stack(
        [parts[2 * b] + parts[2 * b + 1] for b in range(B)], axis=0
    )
    return out, res


def kernel(x, Wk, Wq, Wv, Wp):
    out, _ = _run(x, Wk, Wq, Wv, Wp, trace=False)
    return out


# revision 37
# speedup vs baseline: 1.0327x; 1.0134x over previous
"""Causal self-attention (B=4, T=2048, C=1024, NH=16) on 8 TRN2 NeuronCores.

Sharding: core c -> batch b = c//2, head-group g = c%2 (8 heads, Dh=512).
Each core computes q/k/v projections for its head group on its batch,
fused causal attention (attT layout: k on partitions), and a partial
output projection through its row-slice of Wp. Host sums the two
partials per batch.

Pipeline structure (per 512-row block ib): projections for block ib ->
attention stage jq=ib over all 4 head pairs -> normalization ->
output projection + store for block ib. The Tile list scheduler fills
PE bubbles of the ACT-bound attention chain with projection/out-proj
matmuls of neighboring stages.

Per-core dataflow: everything bf16 on the matmul paths. QK for the
even/odd head of a pair go to the two banks of one [128,1024] psum
tile (row groups 0-1 / 2-3 run concurrently) so ONE activation
instruction exps both. v is stored per-k-chunk as AV lhsT slots with a
ones column that makes softmax denominators a free extra psum row of
the AV matmul. Causal: only lower-triangle k-chunks computed; QK, exp
and AV all trim to the valid q columns; diagonal chunks masked with a
host tri mask after exp. Normalization: s rows collected on 8
partitions of a tiny tile -> one Ln + one Exp -> K=8 PE broadcast
matmul -> one scale per (hp, block).

kernel(**inputs) takes the FULL unsharded inputs and returns the FULL
output. Self-contained: hardcodes all shapes, reads nothing from disk.
"""

import sys

sys.path.insert(0, "/opt/trn_rl_repo")

import numpy as np
import ml_dtypes
from contextlib import ExitStack

import concourse.bass as bass  # noqa: F401  (engine types referenced via nc)
import concourse.mybir as mybir
import concourse.tile as tile
from concourse import bacc
from concourse.bass_utils import run_bass_kernel_spmd

P = 128
B, T, C = 4, 2048, 1024
NH, HS = 16, 64
D = 512          # per-core head dim (8 heads)
H = 8            # local heads
f32 = mybir.dt.float32
bf16 = mybir.dt.bfloat16
AFT = mybir.ActivationFunctionType


def build_nc(t=T):
    """Build the single-core SPMD program (same code, per-core data)."""
    assert t % 512 == 0
    nb = t // 512    # 512-row blocks (proj blocks == q blocks == stages)
    nkc = t // 128   # k chunks of 128
    nco = C // P     # C chunks (8)

    nc = bacc.Bacc("TRN2", target_bir_lowering=False, debug=False, num_devices=8)

    xt_d = nc.dram_tensor("xt", [C, t], bf16, kind="ExternalInput")
    wq_d = nc.dram_tensor("wq", [C, D], bf16, kind="ExternalInput")
    wk_d = nc.dram_tensor("wk", [C, D], bf16, kind="ExternalInput")
    wv_d = nc.dram_tensor("wv", [C, D], bf16, kind="ExternalInput")
    wp_d = nc.dram_tensor("wp", [D, C], bf16, kind="ExternalInput")
    tri_d = nc.dram_tensor("tri", [P, 2, P], bf16, kind="ExternalInput")
    bcm_d = nc.dram_tensor("bcm", [P, 4, P], bf16, kind="ExternalInput")
    out_d = nc.dram_tensor("out", [t, C], bf16, kind="ExternalOutput")

    xt_r = xt_d[:].rearrange("(co p) t -> p co t", p=P)
    wq_r = wq_d[:].rearrange("(co p) d -> p co d", p=P)
    wk_r = wk_d[:].rearrange("(co p) d -> p co d", p=P)
    wv_r = wv_d[:].rearrange("(co p) d -> p co d", p=P)
    wp_r = wp_d[:].rearrange("(dc p) c -> p dc c", p=P)
    out_r = out_d[:].rearrange("(tc p) c -> p tc c", p=P)

    with tile.TileContext(nc) as tc, ExitStack() as ctx, nc.allow_low_precision(
        reason="bf16 attention kernel"
    ):
        perm = ctx.enter_context(tc.tile_pool(name="perm", bufs=1))
        work = ctx.enter_context(tc.tile_pool(name="work", bufs=1))
        psum = ctx.enter_context(tc.tile_pool(name="psum", bufs=1, space="PSUM"))

        # per-block tiles so stage ib's reads and proj ib+1's writes are
        # on different tiles (no false WAR edges to serialize stages)
        qt_t = [perm.tile([P, 4, 512], bf16, name=f"qt{i}") for i in range(nb)]
        kt_t = [perm.tile([P, 4, 512], bf16, name=f"kt{i}") for i in range(nb)]
        v_t = [perm.tile([P, 4, H, P], bf16, name=f"v{i}") for i in range(nb)]
        yt_t = [perm.tile([P, 4, 512], bf16, name=f"yt{i}") for i in range(nb)]
        wq_sb = perm.tile([P, nco, D], bf16)
        wk_sb = perm.tile([P, nco, D], bf16)
        wv_sb = perm.tile([P, nco, D], bf16)
        wp_sb = perm.tile([P, 4, C], bf16)
        tri_sb = perm.tile([P, 2, P], bf16)
        bcm_sb = perm.tile([P, 4, P], bf16)
        # s rows: even head of hp at [64, hp, :], odd at [0, hp, :].
        # Junk partitions 1:64 stay 1.0 so the reciprocal keeps them
        # finite for the K=65 broadcast matmul.
        scol = perm.tile([P, 4, 512], f32)
        scolf = perm.tile([P, 4, 512], f32)
        scolr_t = [perm.tile([P, 4, 512], bf16, name=f"scolr{i}")
                   for i in range(nb)]

        # DMA order: first q-proj matmul needs only chunk 0 of wq and x
        xtb0 = work.tile([P, nco, 512], bf16, tag="xtb", bufs=2)
        nc.sync.dma_start(wq_sb[:, 0, :], wq_r[:, 0, :])
        nc.sync.dma_start(xtb0[:, 0, :], xt_r[:, 0, 0:512])
        nc.sync.dma_start(wq_sb[:, 1:nco, :], wq_r[:, 1:nco, :])
        nc.sync.dma_start(xtb0[:, 1:nco, :], xt_r[:, 1:nco, 0:512])
        nc.sync.dma_start(wk_sb[:], wk_r)
        nc.sync.dma_start(wv_sb[:], wv_r)
        nc.sync.dma_start(tri_sb[:], tri_d[:])
        nc.sync.dma_start(bcm_sb[:], bcm_d[:])
        nc.sync.dma_start(wp_sb[:], wp_r)
        # zero v slots (junk cols would put NaNs in unread psum rows);
        # ones columns: even head -> col 64 (sum row 64); odd -> col 0 (row 0)
        v5s = []
        for vb in v_t:
            nc.gpsimd.memset(vb[:], 0.0)
            v5 = vb[:].rearrange("p k (hp par) c -> p k hp par c", par=2)
            nc.gpsimd.memset(v5[:, :, :, 0, 64:65], 1.0)
            nc.gpsimd.memset(v5[:, :, :, 1, 0:1], 1.0)
            v5s.append(v5)
        nc.gpsimd.memset(scol[:], 1.0)

        def emit_proj(ibb, xtbb):
            for m in range(4):  # qt/kt row chunks of Dh
                psq = psum.tile([P, 512], f32, tag="gen", bufs=2, name="psq")
                for co in range(nco):
                    nc.tensor.matmul(
                        psq[:],
                        wq_sb[:, co, m * P : (m + 1) * P],
                        xtbb[:, co, :],
                        start=(co == 0),
                        stop=(co == nco - 1),
                    )
                nc.vector.tensor_copy(out=qt_t[ibb][:, m, :], in_=psq[:])
                psk = psum.tile([P, 512], f32, tag="gen", bufs=2, name="psk")
                for co in range(nco):
                    nc.tensor.matmul(
                        psk[:],
                        wk_sb[:, co, m * P : (m + 1) * P],
                        xtbb[:, co, :],
                        start=(co == 0),
                        stop=(co == nco - 1),
                    )
                nc.vector.tensor_copy(out=kt_t[ibb][:, m, :], in_=psk[:])
            for t4 in range(4):  # v chunks of 128 rows within this block
                psv = psum.tile([P, 512], f32, tag="gen", bufs=2, name="psv")
                for co in range(nco):
                    nc.tensor.matmul(
                        psv[:],
                        xtbb[:, co, t4 * P : (t4 + 1) * P],
                        wv_sb[:, co, :],
                        start=(co == 0),
                        stop=(co == nco - 1),
                    )
                # scatter heads into lhsT slots: even -> cols 0:64 of
                # slot (par 0), odd -> cols 64:128 (par 1)
                src = psv[:].rearrange("p (hp par c) -> p hp par c", par=2, c=64)
                nc.vector.tensor_copy(
                    out=v5s[ibb][:, t4, :, 0, 0:64], in_=src[:, :, 0, :]
                )
                nc.vector.tensor_copy(
                    out=v5s[ibb][:, t4, :, 1, 64:128], in_=src[:, :, 1, :]
                )

        lo, hi = slice(0, 64), slice(64, 128)

        def emit_att(ibb, fills=()):
            # fills: thunks emitting one deferred out-proj chunk each,
            # injected mid-chain and at head-pair ends so the in-order PE
            # works through them inside this stage's exp-bound stalls
            fills = list(fills)
            nk = (ibb + 1) * 4
            for hp in range(4):
                pav = psum.tile([P, 1024], f32, tag="av", bufs=1, name="pav")
                for kc in range(nk):
                    d = kc - ibb * 4
                    off = 128 * d if d >= 0 else 0
                    ktb, kcl = kt_t[kc // 4], kc % 4
                    pqk = psum.tile([P, 1024], f32, tag="qk", bufs=2, name="pqk")
                    nc.tensor.matmul(
                        pqk[:, off:512],
                        ktb[lo, hp, kcl * P : (kcl + 1) * P],
                        qt_t[ibb][lo, hp, off:512],
                        start=True,
                        stop=True,
                    )
                    nc.tensor.matmul(
                        pqk[:, 512 + off : 1024],
                        ktb[hi, hp, kcl * P : (kcl + 1) * P],
                        qt_t[ibb][hi, hp, off:512],
                        start=True,
                        stop=True,
                    )
                    att = work.tile([P, 1024], bf16, tag="att", bufs=6, name="att")
                    # one exp over both heads; [512, 512+off) is unwritten
                    # psum junk but lands in att cols the trimmed AV never
                    # reads
                    nc.scalar.activation(
                        att[:, off:1024], pqk[:, off:1024], AFT.Exp, scale=0.125
                    )
                    if d >= 0:
                        # one masked mul covers both heads' diagonal blocks
                        # ([P,2,128] view, stride 512); gpsimd keeps it off
                        # the busier DVE
                        att2v = att[:].rearrange("p (two half) -> p two half",
                                                 two=2)
                        nc.gpsimd.tensor_mul(
                            out=att2v[:, :, off : off + P],
                            in0=att2v[:, :, off : off + P],
                            in1=tri_sb[:],
                        )
                    nc.tensor.matmul(
                        pav[:, off:512],
                        v_t[kc // 4][:, kc % 4, 2 * hp, :],
                        att[:, off:512],
                        start=(kc == 0),
                        stop=(kc == nk - 1),
                    )
                    nc.tensor.matmul(
                        pav[:, 512 + off : 1024],
                        v_t[kc // 4][:, kc % 4, 2 * hp + 1, :],
                        att[:, 512 + off : 1024],
                        start=(kc == 0),
                        stop=(kc == nk - 1),
                    )
                    if fills and kc in (nk // 2, (3 * nk) // 4):
                        fills.pop(0)()
                # yt (unnormalized) + s rows out of the psum halves
                nc.vector.tensor_copy(out=yt_t[ibb][lo, hp, :], in_=pav[0:64, 0:512])
                nc.vector.tensor_copy(
                    out=yt_t[ibb][hi, hp, :], in_=pav[64:128, 512:1024]
                )
                nc.vector.tensor_copy(out=scol[64:65, hp, :], in_=pav[64:65, 0:512])
                nc.vector.tensor_copy(out=scol[0:1, hp, :], in_=pav[0:1, 512:1024])
                if fills:
                    fills.pop(0)()

        def emit_recip(ibb):
            # 1/s on DVE; one pass over partitions 0:65 covers both s rows.
            # Off the ACT engine entirely so its table set never leaves exp.
            # Emitted BEFORE the next block's projections so it sits in the
            # DVE queue ahead of their psum casts.
            nc.vector.reciprocal_approx_fast(
                out=scolf[0:65, :, :], in_=scol[0:65, :, :]
            )
            nc.vector.tensor_copy(
                out=scolr_t[ibb][0:65, :, :], in_=scolf[0:65, :, :]
            )

        def emit_normmul(ibb):
            for hp in range(4):
                rb = psum.tile([P, 1024], f32, tag="qk", bufs=2, name="rb")
                nc.tensor.matmul(
                    rb[:, 0:512], bcm_sb[0:65, hp, :], scolr_t[ibb][0:65, hp, :],
                    start=True, stop=True,
                )
                nc.vector.tensor_mul(
                    out=yt_t[ibb][:, hp, :],
                    in0=yt_t[ibb][:, hp, :],
                    in1=rb[:, 0:512],
                )

        def emit_outproj_tcn(ibb, t4):
            # single 128-row out-proj chunk on gen psum slots (free during
            # attention stages) so it never steals the QK double-buffer
            tcn = ibb * 4 + t4
            ob = work.tile([P, C], bf16, tag="ob", bufs=2, name="ob")
            for n2 in range(C // 512):
                pso = psum.tile([P, 512], f32, tag="gen", bufs=2, name="psog")
                for dc in range(4):
                    nc.tensor.matmul(
                        pso[:],
                        yt_t[ibb][:, dc, t4 * P : (t4 + 1) * P],
                        wp_sb[:, dc, n2 * 512 : (n2 + 1) * 512],
                        start=(dc == 0),
                        stop=(dc == 3),
                    )
                nc.vector.tensor_copy(
                    out=ob[:, n2 * 512 : (n2 + 1) * 512], in_=pso[:]
                )
            nc.sync.dma_start(out_r[:, tcn, :], ob[:])

        def emit_outproj(ibb):
            for t4 in range(4):
                tcn = ibb * 4 + t4
                ob = work.tile([P, C], bf16, tag="ob", bufs=2, name="ob")
                pso = psum.tile([P, 1024], f32, tag="qk", bufs=2, name="pso")
                for n2 in range(C // 512):
                    for dc in range(4):
                        nc.tensor.matmul(
                            pso[:, n2 * 512 : (n2 + 1) * 512],
                            yt_t[ibb][:, dc, t4 * P : (t4 + 1) * P],
                            wp_sb[:, dc, n2 * 512 : (n2 + 1) * 512],
                            start=(dc == 0),
                            stop=(dc == 3),
                        )
                nc.vector.tensor_copy(out=ob[:], in_=pso[:])
                nc.sync.dma_start(out_r[:, tcn, :], ob[:])

        # Emission order per stage: attention(ib), then proj(ib+1) so the
        # in-order PE crunches projections while the DVE copy/recip chain
        # finishes, then norm(ib) (ready by then) and out-proj. Blocks 1,2
        # of out-proj are deferred into stage 3's window.
        emit_proj(0, xtb0)
        for ib in range(nb):
            if ib + 1 < nb:
                xtb_next = work.tile([P, nco, 512], bf16, tag="xtb", bufs=2,
                                     name="xtb")
                nc.sync.dma_start(
                    xtb_next[:], xt_r[:, :, (ib + 1) * 512 : (ib + 2) * 512]
                )
            if ib == nb - 1:
                fills = [lambda: emit_normmul(1)] + [
                    (lambda t4=t4: emit_outproj_tcn(1, t4)) for t4 in range(4)
                ] + [lambda: emit_normmul(2)] + [
                    (lambda t4=t4: emit_outproj_tcn(2, t4)) for t4 in range(4)
                ]
            else:
                fills = ()
            emit_att(ib, fills)
            if ib in (1, 2):
                # deferred normmul -> recip can sit behind the next block's
                # projection casts; emitting it after proj pulls the casts
                # that gate the next stage's first QKs up the DVE queue
                emit_proj(ib + 1, xtb_next)
                emit_recip(ib)
            else:
                emit_recip(ib)
                if ib + 1 < nb:
                    emit_proj(ib + 1, xtb_next)
            if ib == 0:
                emit_normmul(0)
                emit_outproj(0)
            elif ib == nb - 1:
                emit_normmul(3)
                emit_outproj(3)

    nc.finalize()
    return nc


_NC = None


def _get_nc():
    global _NC
    if _NC is None:
        _NC = build_nc()
    return _NC


def make_in_maps(x, Wk, Wq, Wv, Wp):
    x = np.asarray(x, dtype=np.float32)
    Wk = np.asarray(Wk, dtype=np.float32)
    Wq = np.asarray(Wq, dtype=np.float32)
    Wv = np.asarray(Wv, dtype=np.float32)
    Wp = np.asarray(Wp, dtype=np.float32)
    tri = np.broadcast_to(
        np.triu(np.ones((P, P), np.float32))[:, None, :], (P, 2, P)
    ).copy().astype(ml_dtypes.bfloat16)
    # bcm broadcasts 1/s rows (even at 64, odd at 0) to yt rows
    bcm = np.zeros((P, 4, P), np.float32)
    for hp in range(4):
        bcm[64, hp, 0:64] = 1.0
        bcm[0, hp, 64:128] = 1.0
    bcm = bcm.astype(ml_dtypes.bfloat16)
    in_maps = []
    for c in range(8):
        b, g = c // 2, c % 2
        sl = slice(g * D, (g + 1) * D)
        in_maps.append({
            "xt": np.ascontiguousarray(x[b].T).astype(ml_dtypes.bfloat16),
            "wq": np.ascontiguousarray(Wq[:, sl]).astype(ml_dtypes.bfloat16),
            "wk": np.ascontiguousarray(Wk[:, sl]).astype(ml_dtypes.bfloat16),
            "wv": np.ascontiguousarray(Wv[:, sl]).astype(ml_dtypes.bfloat16),
            "wp": np.ascontiguousarray(Wp[sl, :]).astype(ml_dtypes.bfloat16),
            "tri": tri,
            "bcm": bcm,
        })
    return in_maps


def _run(x, Wk, Wq, Wv, Wp, trace=False):
    nc = _get_nc()
    in_maps = make_in_maps(x, Wk, Wq, Wv, Wp)
    res = run_bass_kernel_spmd(nc, in_maps, core_ids=list(range(8)), trace=trace)
    parts = [np.asarray(res.results[c]["out"], dtype=np.float32) for c in range(8)]
    out = np.stack(
        [parts[2 * b] + parts[2 * b + 1] for b in range(B)], axis=0
    )
    return out, res


def kernel(x, Wk, Wq, Wv, Wp):
    out, _ = _run(x, Wk, Wq, Wv, Wp, trace=False)
    return out


# revision 38
# speedup vs baseline: 1.0412x; 1.0083x over previous
"""Causal self-attention (B=4, T=2048, C=1024, NH=16) on 8 TRN2 NeuronCores.

Sharding: core c -> batch b = c//2, head-group g = c%2 (8 heads, Dh=512).
Each core computes q/k/v projections for its head group on its batch,
fused causal attention (attT layout: k on partitions), and a partial
output projection through its row-slice of Wp. Host sums the two
partials per batch.

Pipeline structure (per 512-row block ib): projections for block ib ->
attention stage jq=ib over all 4 head pairs -> normalization ->
output projection + store for block ib. The Tile list scheduler fills
PE bubbles of the ACT-bound attention chain with projection/out-proj
matmuls of neighboring stages.

Per-core dataflow: everything bf16 on the matmul paths. QK for the
even/odd head of a pair go to the two banks of one [128,1024] psum
tile (row groups 0-1 / 2-3 run concurrently) so ONE activation
instruction exps both. v is stored per-k-chunk as AV lhsT slots with a
ones column that makes softmax denominators a free extra psum row of
the AV matmul. Causal: only lower-triangle k-chunks computed; QK, exp
and AV all trim to the valid q columns; diagonal chunks masked with a
host tri mask after exp. Normalization: s rows collected on 8
partitions of a tiny tile -> one Ln + one Exp -> K=8 PE broadcast
matmul -> one scale per (hp, block).

kernel(**inputs) takes the FULL unsharded inputs and returns the FULL
output. Self-contained: hardcodes all shapes, reads nothing from disk.
"""

import sys

sys.path.insert(0, "/opt/trn_rl_repo")

import numpy as np
import ml_dtypes
from contextlib import ExitStack

import concourse.bass as bass  # noqa: F401  (engine types referenced via nc)
import concourse.mybir as mybir
import concourse.tile as tile
from concourse import bacc
from concourse.bass_utils import run_bass_kernel_spmd

P = 128
B, T, C = 4, 2048, 1024
NH, HS = 16, 64
D = 512          # per-core head dim (8 heads)
H = 8            # local heads
f32 = mybir.dt.float32
bf16 = mybir.dt.bfloat16
AFT = mybir.ActivationFunctionType


def build_nc(t=T):
    """Build the single-core SPMD program (same code, per-core data)."""
    assert t % 512 == 0
    nb = t // 512    # 512-row blocks (proj blocks == q blocks == stages)
    nkc = t // 128   # k chunks of 128
    nco = C // P     # C chunks (8)

    nc = bacc.Bacc("TRN2", target_bir_lowering=False, debug=False, num_devices=8)

    xt_d = nc.dram_tensor("xt", [C, t], bf16, kind="ExternalInput")
    wq_d = nc.dram_tensor("wq", [C, D], bf16, kind="ExternalInput")
    wk_d = nc.dram_tensor("wk", [C, D], bf16, kind="ExternalInput")
    wv_d = nc.dram_tensor("wv", [C, D], bf16, kind="ExternalInput")
    wp_d = nc.dram_tensor("wp", [D, C], bf16, kind="ExternalInput")
    tri_d = nc.dram_tensor("tri", [P, 2, P], bf16, kind="ExternalInput")
    bcm_d = nc.dram_tensor("bcm", [P, 4, P], bf16, kind="ExternalInput")
    out_d = nc.dram_tensor("out", [t, C], bf16, kind="ExternalOutput")

    xt_r = xt_d[:].rearrange("(co p) t -> p co t", p=P)
    wq_r = wq_d[:].rearrange("(co p) d -> p co d", p=P)
    wk_r = wk_d[:].rearrange("(co p) d -> p co d", p=P)
    wv_r = wv_d[:].rearrange("(co p) d -> p co d", p=P)
    wp_r = wp_d[:].rearrange("(dc p) c -> p dc c", p=P)
    out_r = out_d[:].rearrange("(tc p) c -> p tc c", p=P)

    with tile.TileContext(nc) as tc, ExitStack() as ctx, nc.allow_low_precision(
        reason="bf16 attention kernel"
    ):
        perm = ctx.enter_context(tc.tile_pool(name="perm", bufs=1))
        work = ctx.enter_context(tc.tile_pool(name="work", bufs=1))
        psum = ctx.enter_context(tc.tile_pool(name="psum", bufs=1, space="PSUM"))

        # per-block tiles so stage ib's reads and proj ib+1's writes are
        # on different tiles (no false WAR edges to serialize stages)
        qt_t = [perm.tile([P, 4, 512], bf16, name=f"qt{i}") for i in range(nb)]
        kt_t = [perm.tile([P, 4, 512], bf16, name=f"kt{i}") for i in range(nb)]
        v_t = [perm.tile([P, 4, H, P], bf16, name=f"v{i}") for i in range(nb)]
        yt_t = [perm.tile([P, 4, 512], bf16, name=f"yt{i}") for i in range(nb)]
        wq_sb = perm.tile([P, nco, D], bf16)
        wk_sb = perm.tile([P, nco, D], bf16)
        wv_sb = perm.tile([P, nco, D], bf16)
        wp_sb = perm.tile([P, 4, C], bf16)
        tri_sb = perm.tile([P, 2, P], bf16)
        bcm_sb = perm.tile([P, 4, P], bf16)
        # s rows: even head of hp at [64, hp, :], odd at [0, hp, :].
        # Junk partitions 1:64 stay 1.0 so the reciprocal keeps them
        # finite for the K=65 broadcast matmul.
        scol = perm.tile([P, 4, 512], f32)
        scolf = perm.tile([P, 4, 512], f32)
        scolr_t = [perm.tile([P, 4, 512], bf16, name=f"scolr{i}")
                   for i in range(nb)]

        # DMA order: first q-proj matmul needs only chunk 0 of wq and x
        xtb0 = work.tile([P, nco, 512], bf16, tag="xtb", bufs=2)
        nc.sync.dma_start(wq_sb[:, 0, :], wq_r[:, 0, :])
        nc.sync.dma_start(xtb0[:, 0, :], xt_r[:, 0, 0:512])
        nc.sync.dma_start(wq_sb[:, 1:nco, :], wq_r[:, 1:nco, :])
        nc.sync.dma_start(xtb0[:, 1:nco, :], xt_r[:, 1:nco, 0:512])
        nc.sync.dma_start(wk_sb[:], wk_r)
        nc.sync.dma_start(wv_sb[:], wv_r)
        nc.sync.dma_start(tri_sb[:], tri_d[:])
        nc.sync.dma_start(bcm_sb[:], bcm_d[:])
        nc.sync.dma_start(wp_sb[:], wp_r)
        # zero v slots (junk cols would put NaNs in unread psum rows);
        # ones columns: even head -> col 64 (sum row 64); odd -> col 0 (row 0)
        v5s = []
        for vb in v_t:
            nc.gpsimd.memset(vb[:], 0.0)
            v5 = vb[:].rearrange("p k (hp par) c -> p k hp par c", par=2)
            nc.gpsimd.memset(v5[:, :, :, 0, 64:65], 1.0)
            nc.gpsimd.memset(v5[:, :, :, 1, 0:1], 1.0)
            v5s.append(v5)
        nc.gpsimd.memset(scol[:], 1.0)

        def emit_proj(ibb, xtbb):
            for m in range(4):  # qt/kt row chunks of Dh
                psq = psum.tile([P, 512], f32, tag="gen", bufs=2, name="psq")
                for co in range(nco):
                    nc.tensor.matmul(
                        psq[:],
                        wq_sb[:, co, m * P : (m + 1) * P],
                        xtbb[:, co, :],
                        start=(co == 0),
                        stop=(co == nco - 1),
                    )
                nc.vector.tensor_copy(out=qt_t[ibb][:, m, :], in_=psq[:])
                psk = psum.tile([P, 512], f32, tag="gen", bufs=2, name="psk")
                for co in range(nco):
                    nc.tensor.matmul(
                        psk[:],
                        wk_sb[:, co, m * P : (m + 1) * P],
                        xtbb[:, co, :],
                        start=(co == 0),
                        stop=(co == nco - 1),
                    )
                nc.vector.tensor_copy(out=kt_t[ibb][:, m, :], in_=psk[:])
            for t4 in range(4):  # v chunks of 128 rows within this block
                psv = psum.tile([P, 512], f32, tag="gen", bufs=2, name="psv")
                for co in range(nco):
                    nc.tensor.matmul(
                        psv[:],
                        xtbb[:, co, t4 * P : (t4 + 1) * P],
                        wv_sb[:, co, :],
                        start=(co == 0),
                        stop=(co == nco - 1),
                    )
                # scatter heads into lhsT slots: even -> cols 0:64 of
                # slot (par 0), odd -> cols 64:128 (par 1)
                src = psv[:].rearrange("p (hp par c) -> p hp par c", par=2, c=64)
                nc.vector.tensor_copy(
                    out=v5s[ibb][:, t4, :, 0, 0:64], in_=src[:, :, 0, :]
                )
                nc.vector.tensor_copy(
                    out=v5s[ibb][:, t4, :, 1, 64:128], in_=src[:, :, 1, :]
                )

        lo, hi = slice(0, 64), slice(64, 128)

        def emit_att(ibb, fills=()):
            # fills: thunks emitting one deferred out-proj chunk each,
            # injected mid-chain and at head-pair ends so the in-order PE
            # works through them inside this stage's exp-bound stalls
            fills = list(fills)
            nk = (ibb + 1) * 4
            for hp in range(4):
                pav = psum.tile([P, 1024], f32, tag="av", bufs=1, name="pav")
                for kc in range(nk):
                    d = kc - ibb * 4
                    off = 128 * d if d >= 0 else 0
                    ktb, kcl = kt_t[kc // 4], kc % 4
                    pqk = psum.tile([P, 1024], f32, tag="qk", bufs=2, name="pqk")
                    nc.tensor.matmul(
                        pqk[:, off:512],
                        ktb[lo, hp, kcl * P : (kcl + 1) * P],
                        qt_t[ibb][lo, hp, off:512],
                        start=True,
                        stop=True,
                    )
                    nc.tensor.matmul(
                        pqk[:, 512 + off : 1024],
                        ktb[hi, hp, kcl * P : (kcl + 1) * P],
                        qt_t[ibb][hi, hp, off:512],
                        start=True,
                        stop=True,
                    )
                    att = work.tile([P, 1024], bf16, tag="att", bufs=6, name="att")
                    # one exp over both heads; [512, 512+off) is unwritten
                    # psum junk but lands in att cols the trimmed AV never
                    # reads
                    nc.scalar.activation(
                        att[:, off:1024], pqk[:, off:1024], AFT.Exp, scale=0.125
                    )
                    if d >= 0:
                        # one masked mul covers both heads' diagonal blocks
                        # ([P,2,128] view, stride 512); gpsimd keeps it off
                        # the busier DVE
                        att2v = att[:].rearrange("p (two half) -> p two half",
                                                 two=2)
                        nc.gpsimd.tensor_mul(
                            out=att2v[:, :, off : off + P],
                            in0=att2v[:, :, off : off + P],
                            in1=tri_sb[:],
                        )
                    nc.tensor.matmul(
                        pav[:, off:512],
                        v_t[kc // 4][:, kc % 4, 2 * hp, :],
                        att[:, off:512],
                        start=(kc == 0),
                        stop=(kc == nk - 1),
                    )
                    nc.tensor.matmul(
                        pav[:, 512 + off : 1024],
                        v_t[kc // 4][:, kc % 4, 2 * hp + 1, :],
                        att[:, 512 + off : 1024],
                        start=(kc == 0),
                        stop=(kc == nk - 1),
                    )
                    if fills and kc in (nk // 2, (3 * nk) // 4):
                        fills.pop(0)()
                # yt (unnormalized) + s rows out of the psum halves
                nc.vector.tensor_copy(out=yt_t[ibb][lo, hp, :], in_=pav[0:64, 0:512])
                nc.vector.tensor_copy(
                    out=yt_t[ibb][hi, hp, :], in_=pav[64:128, 512:1024]
                )
                nc.vector.tensor_copy(out=scol[64:65, hp, :], in_=pav[64:65, 0:512])
                nc.vector.tensor_copy(out=scol[0:1, hp, :], in_=pav[0:1, 512:1024])
                if fills:
                    fills.pop(0)()

        def emit_recip(ibb):
            # 1/s on DVE; one pass over partitions 0:65 covers both s rows.
            # Off the ACT engine entirely so its table set never leaves exp.
            # Emitted BEFORE the next block's projections so it sits in the
            # DVE queue ahead of their psum casts.
            nc.vector.reciprocal_approx_fast(
                out=scolf[0:65, :, :], in_=scol[0:65, :, :]
            )
            nc.vector.tensor_copy(
                out=scolr_t[ibb][0:65, :, :], in_=scolf[0:65, :, :]
            )

        def emit_normmul(ibb):
            for hp in range(4):
                rb = psum.tile([P, 1024], f32, tag="qk", bufs=2, name="rb")
                nc.tensor.matmul(
                    rb[:, 0:512], bcm_sb[0:65, hp, :], scolr_t[ibb][0:65, hp, :],
                    start=True, stop=True,
                )
                nc.vector.tensor_mul(
                    out=yt_t[ibb][:, hp, :],
                    in0=yt_t[ibb][:, hp, :],
                    in1=rb[:, 0:512],
                )

        def emit_outproj_tcn(ibb, t4):
            # single 128-row out-proj chunk on gen psum slots (free during
            # attention stages) so it never steals the QK double-buffer
            tcn = ibb * 4 + t4
            ob = work.tile([P, C], bf16, tag="ob", bufs=2, name="ob")
            for n2 in range(C // 512):
                pso = psum.tile([P, 512], f32, tag="gen", bufs=2, name="psog")
                for dc in range(4):
                    nc.tensor.matmul(
                        pso[:],
                        yt_t[ibb][:, dc, t4 * P : (t4 + 1) * P],
                        wp_sb[:, dc, n2 * 512 : (n2 + 1) * 512],
                        start=(dc == 0),
                        stop=(dc == 3),
                    )
                nc.vector.tensor_copy(
                    out=ob[:, n2 * 512 : (n2 + 1) * 512], in_=pso[:]
                )
            nc.sync.dma_start(out_r[:, tcn, :], ob[:])

        def emit_outproj(ibb):
            for t4 in range(4):
                tcn = ibb * 4 + t4
                ob = work.tile([P, C], bf16, tag="ob", bufs=2, name="ob")
                pso = psum.tile([P, 1024], f32, tag="qk", bufs=2, name="pso")
                for n2 in range(C // 512):
                    for dc in range(4):
                        nc.tensor.matmul(
                            pso[:, n2 * 512 : (n2 + 1) * 512],
                            yt_t[ibb][:, dc, t4 * P : (t4 + 1) * P],
                            wp_sb[:, dc, n2 * 512 : (n2 + 1) * 512],
                            start=(dc == 0),
                            stop=(dc == 3),
                        )
                nc.vector.tensor_copy(out=ob[:], in_=pso[:])
                nc.sync.dma_start(out_r[:, tcn, :], ob[:])

        # Emission order per stage: attention(ib), then proj(ib+1) so the
        # in-order PE crunches projections while the DVE copy/recip chain
        # finishes, then norm(ib) (ready by then) and out-proj. Blocks 1,2
        # of out-proj are deferred into stage 3's window.
        emit_proj(0, xtb0)
        for ib in range(nb):
            if ib + 1 < nb:
                xtb_next = work.tile([P, nco, 512], bf16, tag="xtb", bufs=2,
                                     name="xtb")
                nc.sync.dma_start(
                    xtb_next[:], xt_r[:, :, (ib + 1) * 512 : (ib + 2) * 512]
                )
            if ib == nb - 1:
                # last two chunks stay out of the fill list: emitted right
                # after att3 they keep the PE busy under the tail DVE chain
                fills = [lambda: emit_normmul(1)] + [
                    (lambda t4=t4: emit_outproj_tcn(1, t4)) for t4 in range(4)
                ] + [lambda: emit_normmul(2)] + [
                    (lambda t4=t4: emit_outproj_tcn(2, t4)) for t4 in range(2)
                ]
            else:
                fills = ()
            emit_att(ib, fills)
            if ib == nb - 1:
                emit_outproj_tcn(2, 2)
                emit_outproj_tcn(2, 3)
            if ib in (1, 2):
                # deferred normmul -> recip can sit behind the next block's
                # projection casts; emitting it after proj pulls the casts
                # that gate the next stage's first QKs up the DVE queue
                emit_proj(ib + 1, xtb_next)
                emit_recip(ib)
            else:
                emit_recip(ib)
                if ib + 1 < nb:
                    emit_proj(ib + 1, xtb_next)
            if ib == 0:
                emit_normmul(0)
                emit_outproj(0)
            elif ib == nb - 1:
                emit_normmul(3)
                emit_outproj(3)

    nc.finalize()
    return nc


_NC = None


def _get_nc():
    global _NC
    if _NC is None:
        _NC = build_nc()
    return _NC


def make_in_maps(x, Wk, Wq, Wv, Wp):
    x = np.asarray(x, dtype=np.float32)
    Wk = np.asarray(Wk, dtype=np.float32)
    Wq = np.asarray(Wq, dtype=np.float32)
    Wv = np.asarray(Wv, dtype=np.float32)
    Wp = np.asarray(Wp, dtype=np.float32)
    tri = np.broadcast_to(
        np.triu(np.ones((P, P), np.float32))[:, None, :], (P, 2, P)
    ).copy().astype(ml_dtypes.bfloat16)
    # bcm broadcasts 1/s rows (even at 64, odd at 0) to yt rows
    bcm = np.zeros((P, 4, P), np.float32)
    for hp in range(4):
        bcm[64, hp, 0:64] = 1.0
        bcm[0, hp, 64:128] = 1.0
    bcm = bcm.astype(ml_dtypes.bfloat16)
    in_maps = []
    for c in range(8):
        b, g = c // 2, c % 2
        sl = slice(g * D, (g + 1) * D)
        in_maps.append({
            "xt": np.ascontiguousarray(x[b].T).astype(ml_dtypes.bfloat16),
            "wq": np.ascontiguousarray(Wq[:, sl]).astype(ml_dtypes.bfloat16),
            "wk": np.ascontiguousarray(Wk[:, sl]).astype(ml_dtypes.bfloat16),
            "wv": np.ascontiguousarray(Wv[:, sl]).astype(ml_dtypes.bfloat16),
            "wp": np.ascontiguousarray(Wp[sl, :]).astype(ml_dtypes.bfloat16),
            "tri": tri,
            "bcm": bcm,
        })
    return in_maps


def _run(x, Wk, Wq, Wv, Wp, trace=False):
    nc = _get_nc()
    in_maps = make_in_maps(x, Wk, Wq, Wv, Wp)
    res = run_bass_kernel_spmd(nc, in_maps, core_ids=list(range(8)), trace=trace)
    parts = [np.asarray(res.results[c]["out"], dtype=np.float32) for c in range(8)]
    out = np.stack(
        [parts[2 * b] + parts[2 * b + 1] for b in range(B)], axis=0
    )
    return out, res


def kernel(x, Wk, Wq, Wv, Wp):
    out, _ = _run(x, Wk, Wq, Wv, Wp, trace=False)
    return out
